# revision 2
# baseline (speedup 1.0000x reference)
"""Trainium2 Bass kernel v2 for nn_Attention_42288247996512.

reference:
  q = x @ Wq.T; k = cross @ Wk.T; v = x @ Wv.T
  logits = q @ k.T  (causal; padding m_q*m_k, diag always kept)
  out = softmax(logits / sqrt(128)) @ v

v2 refactor vs baseline:
  * out^T = Wv @ (x^T @ attn^T): the value projection moves to the query
    side (65536 PE rows/core) instead of projecting v for all 2048 keys
    (131072 rows). The AX stage (x^T @ attn^T) consumes raw x in natural
    layout as lhsT -- no projection needed before it.
  * all matmuls in bf16 (1.0 PE cycles/row at any ap size, transposes
    1.0 vs f32r's 1.5), all DMA payloads bf16 (halves HBM traffic).
  * strips interleave even/odd key blocks so both strips share exact
    per-slot causal widths W[s] = 256*(s+1): logits/exp/transpose/AX run
    only over live key blocks (73728 AX rows vs 81920 for 512-chunking).
  * single SP HWDGE FIFO carries inputs in consumption order; outputs
    ride the ACT HWDGE queue so they never block input delivery.

Per-core PE rows: qT 8192 + kT 16384 + logits 9216 + transp 9216 +
AX 73728 + proj 65536 = 182272 (~76us at peak rate) vs ~263k in v1.
"""
import math
import threading

import ml_dtypes
import numpy as np

B, S, D, DA = 4, 2048, 1024, 128
P = 128
KC = D // P  # 8 contraction chunks
NCORES = 8
NQ = 1024
BIG = 32768.0  # power of two: exactly representable in bf16

# strip p holds key blocks STRIPS[p][s] at slot s; even/odd interleave
# makes the per-slot causal width identical across strips:
# W[s] = (max(2s, 2s+1) + 1) * 128 = 256 * (s + 1)
STRIPS = [[0, 2, 4, 6, 8, 10, 12, 14], [1, 3, 5, 7, 9, 11, 13, 15]]

_BUILD_LOCK = threading.Lock()
_CACHE: dict = {}


def _build():
    from contextlib import ExitStack

    import concourse.mybir as mybir
    import concourse.tile as tile
    from concourse import bacc
    from concourse.masks import make_identity

    dt = mybir.dt
    f32 = dt.float32
    bf16 = dt.bfloat16
    AF = mybir.ActivationFunctionType
    ALU = mybir.AluOpType

    nc = bacc.Bacc("TRN2", target_bir_lowering=False, debug=False)

    # all inputs pre-rearranged on host so every DMA source row is
    # partition-contiguous (2KB+ descriptors run at ~360B/ns; the naive
    # (kc p) m -> p kc m rearranges produce 256B descriptors at ~172B/ns)
    xb = nc.dram_tensor("xb", [S, D], bf16, kind="ExternalInput").ap()
    ct_d = nc.dram_tensor("ct", [P, 4, KC, 512], bf16, kind="ExternalInput").ap()
    xq_d = nc.dram_tensor("xq", [P, 4, KC, 256], bf16, kind="ExternalInput").ap()
    wq_d = nc.dram_tensor("wq", [P, KC, DA], bf16, kind="ExternalInput").ap()
    wk_d = nc.dram_tensor("wk", [P, KC, DA], bf16, kind="ExternalInput").ap()
    wv_d = nc.dram_tensor("wv", [P, 2, KC, 512], bf16, kind="ExternalInput").ap()
    kmb = nc.dram_tensor("kmb", [P, 1536], bf16, kind="ExternalInput").ap()
    qmn = nc.dram_tensor("qmn", [P, 8], f32, kind="ExternalInput").ap()
    # packed per-slot final-chunk masks: even slots are 256 wide, odd 512
    dmask = nc.dram_tensor("dmask", [P, 3072], bf16, kind="ExternalInput").ap()

    outT = nc.dram_tensor("outT", [D, NQ], bf16, kind="ExternalOutput").ap()
    den = nc.dram_tensor("den", [P, 8], f32, kind="ExternalOutput").ap()

    xb_r = xb.rearrange("(g p) d -> p g d", p=P)
    outT_r = outT.rearrange("(do p) q -> p do q", p=P)

    with tile.TileContext(nc) as tc, ExitStack() as ctx:
        const = ctx.enter_context(tc.tile_pool(name="const", bufs=1))
        persist = ctx.enter_context(tc.tile_pool(name="persist", bufs=1))
        stream = ctx.enter_context(tc.tile_pool(name="stream", bufs=2))
        apool = ctx.enter_context(tc.tile_pool(name="apool", bufs=4))
        epool = ctx.enter_context(tc.tile_pool(name="epool", bufs=32))
        psl_pool = ctx.enter_context(tc.tile_pool(name="psl", bufs=2, space="PSUM"))
        psT_pool = ctx.enter_context(tc.tile_pool(name="psT", bufs=2, space="PSUM"))
        psax_pool = ctx.enter_context(tc.tile_pool(name="psax", bufs=2, space="PSUM"))
        pjp_pool = ctx.enter_context(tc.tile_pool(name="pjp", bufs=2, space="PSUM"))

        ident_f32 = const.tile([P, P], f32, name="ident_f32")
        make_identity(nc, ident_f32)
        ident = const.tile([P, P], bf16, name="ident")
        nc.vector.tensor_copy(ident[:], ident_f32[:])

        wq_sb = const.tile([P, KC, DA], bf16, name="wq_sb")
        wk_sb = const.tile([P, KC, DA], bf16, name="wk_sb")
        wv_sb = const.tile([P, 2, KC, 512], bf16, name="wv_sb")
        kmb_sb = const.tile([P, 1536], bf16, name="kmb_sb")
        qmn_sb = const.tile([P, 8], f32, name="qmn_sb")
        dm_sb = const.tile([P, 3072], bf16, name="dm_sb")
        # packed column offsets of slot s's final-chunk mask in dm_sb
        DOFF = [0, 256, 768, 1024, 1536, 1792, 2304, 2560]

        kT_sb = persist.tile([P, S], bf16, name="kT_sb")
        qT_sb = persist.tile([P, NQ], bf16, name="qT_sb")
        xb_sb = persist.tile([P, 16, D], bf16, name="xb_sb")
        den_sb = persist.tile([P, 8], f32, name="den_sb")

        es: dict = {}
        dacs: dict = {s: [] for s in range(8)}

        def qT_quarter(qtr, nsub=1):
            xq = stream.tile([P, KC, 256], bf16, tag="xq", name=f"xq{qtr}")
            kcs = KC // nsub
            for sub in range(nsub):
                if qtr == 0:
                    nc.sync.dma_start(
                        wq_sb[:, sub * kcs:(sub + 1) * kcs, :],
                        wq_d[:, sub * kcs:(sub + 1) * kcs, :])
                nc.sync.dma_start(
                    xq[:, sub * kcs:(sub + 1) * kcs, :],
                    xq_d[:, qtr, sub * kcs:(sub + 1) * kcs, :])
            ps = pjp_pool.tile([P, 512], f32, tag="pjp", name=f"psq{qtr}")
            for kc in range(KC):
                nc.tensor.matmul(
                    ps[:, :256],
                    lhsT=wq_sb[:, kc, :],
                    rhs=xq[:, kc, :],
                    start=(kc == 0), stop=(kc == KC - 1))
            nc.any.tensor_copy(qT_sb[:, qtr * 256:(qtr + 1) * 256], ps[:, :256])

        def kT_chunk(j, nsub=1):
            ct = stream.tile([P, KC, 512], bf16, tag="ct", name=f"ct{j}")
            kcs = KC // nsub
            for sub in range(nsub):
                if j == 0:
                    nc.sync.dma_start(
                        wk_sb[:, sub * kcs:(sub + 1) * kcs, :],
                        wk_d[:, sub * kcs:(sub + 1) * kcs, :])
                nc.sync.dma_start(
                    ct[:, sub * kcs:(sub + 1) * kcs, :],
                    ct_d[:, j, sub * kcs:(sub + 1) * kcs, :])
            ps = pjp_pool.tile([P, 512], f32, tag="pjp", name=f"psk{j}")
            for kc in range(KC):
                nc.tensor.matmul(
                    ps[:],
                    lhsT=wk_sb[:, kc, :],
                    rhs=ct[:, kc, :],
                    start=(kc == 0), stop=(kc == KC - 1))
            nc.any.tensor_copy(kT_sb[:, j * 512:(j + 1) * 512], ps[:])

        def pair_logits(pr):
            for j in range(pr + 1):
                for s in (2 * pr, 2 * pr + 1):
                    last = (j == pr)
                    w = 256 if (last and s % 2 == 0) else 512
                    off = j * 512
                    psl = psl_pool.tile([P, 512], f32, tag="psl",
                                        name=f"psl{s}_{j}")
                    nc.tensor.matmul(
                        psl[:, :w],
                        lhsT=qT_sb[:, s * P:(s + 1) * P],
                        rhs=kT_sb[:, off:off + w],
                        start=True, stop=True)
                    sbl = apool.tile([P, 512], f32, tag="sbl",
                                     name=f"sbl{s}_{j}", bufs=8)
                    msrc = dm_sb[:, DOFF[s]:DOFF[s] + w] if last \
                        else kmb_sb[:, off:off + w]
                    nc.vector.tensor_tensor(
                        out=sbl[:, :w], in0=psl[:, :w], in1=msrc, op=ALU.add)
                    e = apool.tile([P, 512], bf16, tag="e",
                                   name=f"e{s}_{j}", bufs=10)
                    dac = apool.tile([P, 1], f32, tag="dac",
                                     name=f"dac{s}_{j}", bufs=12)
                    nc.scalar.activation(
                        e[:, :w], sbl[:, :w], AF.Exp,
                        bias=qmn_sb[:, s:s + 1], scale=1.0,
                        accum_out=dac[:])
                    es[(s, j)] = e
                    dacs[s].append(dac)

        def pair_transp(pr):
            eTs = []
            s0, s1 = 2 * pr, 2 * pr + 1
            for j in range(pr + 1):
                last = (j == pr)
                for ks in range(4):
                    kb = 4 * j + ks
                    both = (not last) or (ks < 2)
                    psT = psT_pool.tile([P, 256], bf16, tag="psT",
                                        name=f"psT{pr}_{kb}")
                    if both:
                        nc.tensor.transpose(
                            psT[:, 0:P], es[(s0, j)][:, ks * P:(ks + 1) * P],
                            ident[:])
                        nc.tensor.transpose(
                            psT[:, P:2 * P], es[(s1, j)][:, ks * P:(ks + 1) * P],
                            ident[:])
                        w = 256
                    else:
                        nc.tensor.transpose(
                            psT[:, 0:P], es[(s1, j)][:, ks * P:(ks + 1) * P],
                            ident[:])
                        w = 128
                    eT = epool.tile([P, 256], bf16, tag="eT",
                                    name=f"eT{pr}_{kb}")
                    nc.any.tensor_copy(eT[:, :w], psT[:, :w])
                    eTs.append((eT, both))
            return eTs

        def pair_ax(pr, eTs):
            nkb0 = 4 * pr + 2
            nkb1 = 4 * pr + 4
            zT = apool.tile([P, KC, 256], bf16, tag="zT", name=f"zT{pr}",
                            bufs=4)
            # two dm-chunks share one [128,512] bank as a single
            # accumulation group (start lazily zeroes the whole 2KB
            # region), so one copy drains two dm-chunks: the copy
            # turnaround (~1us with sem latency) stays ahead of the
            # ~1.3us 2-chunk matmul stream on 2 rotating banks
            for dmh in range(KC // 2):
                ps = psax_pool.tile([P, 512], f32, tag="psax",
                                    name=f"psax{pr}_{dmh}")
                for half in range(2):
                    dmc = dmh * 2 + half
                    base = half * 256
                    for kb in range(nkb1):
                        eT, both = eTs[kb]
                        lhsT = xb_sb[:, kb, dmc * P:(dmc + 1) * P]
                        first = (kb == 0 and half == 0)
                        last = (kb == nkb1 - 1 and half == 1)
                        if both:
                            nc.tensor.matmul(
                                ps[:, base:base + P], lhsT=lhsT,
                                rhs=eT[:, 0:P],
                                start=first, stop=False)
                            nc.tensor.matmul(
                                ps[:, base + P:base + 2 * P], lhsT=lhsT,
                                rhs=eT[:, P:2 * P],
                                start=False, stop=False)
                        else:
                            nc.tensor.matmul(
                                ps[:, base + P:base + 2 * P], lhsT=lhsT,
                                rhs=eT[:, 0:P],
                                start=False, stop=last)
                nc.any.tensor_copy(zT[:, dmh * 2:dmh * 2 + 2, :], ps[:])
            return zT

        def pair_proj(pr, zT):
            osb = apool.tile([P, KC, 256], bf16, tag="osb", name=f"osb{pr}",
                             bufs=2)
            # outputs ride the ACT HWDGE queue: the SP input FIFO never
            # stalls behind an output whose source copy isn't done yet.
            # The last pair flushes per dout so the final DMA after the
            # last matmul is only 64KB.
            ndma = 8 if pr == 3 else 1
            dos = KC // ndma
            for do in range(KC):
                ps = pjp_pool.tile([P, 512], f32, tag="pjp",
                                   name=f"psp{pr}_{do}")
                for dmc in range(KC):
                    nc.tensor.matmul(
                        ps[:, :256],
                        lhsT=wv_sb[:, do // 4, dmc,
                                   (do % 4) * P:(do % 4 + 1) * P],
                        rhs=zT[:, dmc, :],
                        start=(dmc == 0), stop=(dmc == KC - 1))
                nc.any.tensor_copy(osb[:, do, :], ps[:, :256])
                if do % dos == dos - 1:
                    g0 = do - dos + 1
                    nc.scalar.dma_start(
                        outT_r[:, g0:do + 1, pr * 256:(pr + 1) * 256],
                        osb[:, g0:do + 1, :])

        def pair_den(pr):
            for s in (2 * pr, 2 * pr + 1):
                dl = dacs[s]
                dst = den_sb[:, s:s + 1]
                if len(dl) == 1:
                    nc.vector.tensor_copy(dst, dl[0][:])
                else:
                    nc.vector.tensor_tensor(out=dst, in0=dl[0][:],
                                            in1=dl[1][:], op=ALU.add)
                    for d in dl[2:]:
                        nc.vector.tensor_tensor(out=dst, in0=dst, in1=d[:],
                                                op=ALU.add)

        # ---- schedule ----
        # Software-pipelined PE stream; SP FIFO carries inputs in exactly
        # the PE consumption order:
        #   kT0 qT0 log0 qT1 T0 | kT1 log1 AX0 qT2 T1 | kT2 log2 AX1 qT3
        #   T2 | kT3 log3 AX2 T3 AX3 | proj0..3
        # AX(pr-1) + qT(pr+1) fill the exp(pr) DVE+ACT latency before
        # T(pr); all Wv projections run dependency-free at the tail.
        def dm_piece(pr):
            nc.sync.dma_start(dm_sb[:, 768 * pr:768 * (pr + 1)],
                              dmask[:, 768 * pr:768 * (pr + 1)])

        zTs = {}
        eT_all = {}

        kT_chunk(0, nsub=2)
        qT_quarter(0, nsub=2)
        nc.sync.dma_start(qmn_sb[:], qmn)
        dm_piece(0)
        pair_logits(0)
        qT_quarter(1)
        eT_all[0] = pair_transp(0)
        pair_den(0)

        kT_chunk(1)
        nc.sync.dma_start(xb_sb[:, 0:4, :], xb_r[:, 0:4, :])
        nc.sync.dma_start(kmb_sb[:], kmb)
        dm_piece(1)
        pair_logits(1)
        zTs[0] = pair_ax(0, eT_all[0])
        qT_quarter(2)
        eT_all[1] = pair_transp(1)
        pair_den(1)

        kT_chunk(2)
        nc.sync.dma_start(xb_sb[:, 4:8, :], xb_r[:, 4:8, :])
        dm_piece(2)
        pair_logits(2)
        zTs[1] = pair_ax(1, eT_all[1])
        qT_quarter(3)
        eT_all[2] = pair_transp(2)
        pair_den(2)

        kT_chunk(3)
        nc.sync.dma_start(xb_sb[:, 8:12, :], xb_r[:, 8:12, :])
        dm_piece(3)
        nc.sync.dma_start(xb_sb[:, 12:16, :], xb_r[:, 12:16, :])
        # wv is only consumed by the tail projections; keeping it out of
        # the front window lets xb/ct/xq land sooner
        nc.sync.dma_start(wv_sb[:, 0], wv_d[:, 0])
        nc.sync.dma_start(wv_sb[:, 1], wv_d[:, 1])
        pair_logits(3)
        zTs[2] = pair_ax(2, eT_all[2])
        eT_all[3] = pair_transp(3)
        pair_den(3)
        # den is final here; flush it during the tail work instead of
        # after the last output piece
        nc.scalar.dma_start(den[:], den_sb[:])
        zTs[3] = pair_ax(3, eT_all[3])
        for pr in range(4):
            pair_proj(pr, zTs[pr])

    nc.compile()
    return nc


def _get_nc():
    with _BUILD_LOCK:
        if "nc" not in _CACHE:
            _CACHE["nc"] = _build()
        return _CACHE["nc"]


def make_in_maps(x, cross, Wq, Wk, Wv, mask):
    bf = ml_dtypes.bfloat16
    x = np.asarray(x, dtype=np.float32)
    cross = np.asarray(cross, dtype=np.float32)
    scale = 1.0 / math.sqrt(DA)
    # weight layouts pre-rearranged for partition-contiguous DMA rows
    # wq/wk: [dm, da] -> [p, kc, da]  (dm = kc*128 + p)
    wq_h = np.ascontiguousarray(
        (np.asarray(Wq, np.float32) * scale).T.reshape(KC, P, DA)
        .transpose(1, 0, 2)).astype(bf)
    wk_h = np.ascontiguousarray(
        np.asarray(Wk, np.float32).T.reshape(KC, P, DA)
        .transpose(1, 0, 2)).astype(bf)
    # wv: [dm, dout] -> [p, half, kc, 512]
    wv_h = np.ascontiguousarray(
        np.asarray(Wv, np.float32).T.reshape(KC, P, 2, 512)
        .transpose(1, 2, 0, 3)).astype(bf)
    mf = np.asarray(mask).astype(np.float32)  # [B, S]

    karange = np.arange(S)
    in_maps = []
    rows_per_core = []
    for core in range(NCORES):
        b, p = divmod(core, 2)
        blocks = STRIPS[p]
        rows = np.concatenate([np.arange(g * P, (g + 1) * P) for g in blocks])
        rows_per_core.append((b, rows))
        mb = mf[b]
        kneg = (-BIG * (1.0 - mb)).astype(np.float32)  # [S]
        kmb_h = np.ascontiguousarray(
            np.broadcast_to(kneg[:1536], (P, 1536))).astype(bf)
        mq = mb[rows]  # [1024]
        qmn_h = np.ascontiguousarray(
            (-BIG * (1.0 - mq)).reshape(8, P).T)  # [128, 8]
        # packed per-slot final-chunk masks (even slots 256 wide, odd 512)
        doff = [0, 256, 768, 1024, 1536, 1792, 2304, 2560]
        dm_h = np.zeros((P, 3072), np.float32)
        for s, g in enumerate(blocks):
            pr = s // 2
            k0 = 512 * pr
            w = 256 if s % 2 == 0 else 512
            kk = karange[k0:k0 + w]
            qq = g * P + np.arange(P)
            mqs = mq[s * P:(s + 1) * P]
            t = np.broadcast_to(kneg[k0:k0 + w], (P, w)).copy()
            t += -BIG * (kk[None, :] > qq[:, None])
            # diagonal is ALWAYS kept (reference adds eye before the >0
            # test): cancel qmn's -BIG for masked q rows, zero otherwise
            dd = kk[None, :] == qq[:, None]
            t = np.where(dd, (BIG * (1.0 - mqs))[:, None], t)
            dm_h[:, doff[s]:doff[s] + w] = t
        # cross^T: [dm, key] -> [p, chunk_j, kc, 512]
        ct_h = np.ascontiguousarray(
            cross[b].T.reshape(KC, P, 4, 512).transpose(1, 2, 0, 3)).astype(bf)
        # x[rows]^T: [dm, q] -> [p, qtr, kc, 256]
        xq_h = np.ascontiguousarray(
            x[b][rows].T.reshape(KC, P, 4, 256).transpose(1, 2, 0, 3)).astype(bf)
        in_maps.append({
            "xb": np.ascontiguousarray(x[b]).astype(bf),
            "ct": ct_h,
            "xq": xq_h,
            "wq": wq_h,
            "wk": wk_h,
            "wv": wv_h,
            "kmb": kmb_h,
            "qmn": qmn_h,
            "dmask": dm_h.astype(bf),
        })
    return in_maps, rows_per_core


def kernel(x, cross, Wq, Wk, Wv, mask):
    from concourse import bass_utils

    nc = _get_nc()
    in_maps, rows_per_core = make_in_maps(x, cross, Wq, Wk, Wv, mask)
    res = bass_utils.run_bass_kernel_spmd(
        nc, in_maps, core_ids=list(range(NCORES)))

    out = np.empty((B, S, D), np.float32)
    for core in range(NCORES):
        b, rows = rows_per_core[core]
        r = res.results[core]
        o = r["outT"].astype(np.float32).T  # [1024 q, 1024 dm]
        denf = r["den"].T.reshape(-1).astype(np.float32)  # [1024] slot-major
        out[b, rows] = o / denf[:, None]
    return out


# revision 4
# speedup vs baseline: 1.0116x; 1.0116x over previous
"""Trainium2 Bass kernel v2 for nn_Attention_42288247996512.

reference:
  q = x @ Wq.T; k = cross @ Wk.T; v = x @ Wv.T
  logits = q @ k.T  (causal; padding m_q*m_k, diag always kept)
  out = softmax(logits / sqrt(128)) @ v

v2 refactor vs baseline:
  * out^T = Wv @ (x^T @ attn^T): the value projection moves to the query
    side (65536 PE rows/core) instead of projecting v for all 2048 keys
    (131072 rows). The AX stage (x^T @ attn^T) consumes raw x in natural
    layout as lhsT -- no projection needed before it.
  * all matmuls in bf16 (1.0 PE cycles/row at any ap size, transposes
    1.0 vs f32r's 1.5), all DMA payloads bf16 (halves HBM traffic).
  * strips interleave even/odd key blocks so both strips share exact
    per-slot causal widths W[s] = 256*(s+1): logits/exp/transpose/AX run
    only over live key blocks (73728 AX rows vs 81920 for 512-chunking).
  * single SP HWDGE FIFO carries inputs in consumption order; outputs
    ride the ACT HWDGE queue so they never block input delivery.

Per-core PE rows: qT 8192 + kT 16384 + logits 9216 + transp 9216 +
AX 73728 + proj 65536 = 182272 (~76us at peak rate) vs ~263k in v1.
"""
import math
import threading

import ml_dtypes
import numpy as np

B, S, D, DA = 4, 2048, 1024, 128
P = 128
KC = D // P  # 8 contraction chunks
NCORES = 8
NQ = 1024
BIG = 32768.0  # power of two: exactly representable in bf16

# strip p holds key blocks STRIPS[p][s] at slot s; even/odd interleave
# makes the per-slot causal width identical across strips:
# W[s] = (max(2s, 2s+1) + 1) * 128 = 256 * (s + 1)
STRIPS = [[0, 2, 4, 6, 8, 10, 12, 14], [1, 3, 5, 7, 9, 11, 13, 15]]

_BUILD_LOCK = threading.Lock()
_CACHE: dict = {}


def _build():
    from contextlib import ExitStack

    import concourse.mybir as mybir
    import concourse.tile as tile
    from concourse import bacc
    from concourse.masks import make_identity

    dt = mybir.dt
    f32 = dt.float32
    bf16 = dt.bfloat16
    AF = mybir.ActivationFunctionType
    ALU = mybir.AluOpType

    nc = bacc.Bacc("TRN2", target_bir_lowering=False, debug=False)

    # all inputs pre-rearranged on host so every DMA source row is
    # partition-contiguous (2KB+ descriptors run at ~360B/ns; the naive
    # (kc p) m -> p kc m rearranges produce 256B descriptors at ~172B/ns)
    xb = nc.dram_tensor("xb", [S, D], bf16, kind="ExternalInput").ap()

    ct_d = nc.dram_tensor("ct", [P, 4, KC, 512], bf16, kind="ExternalInput").ap()
    xq_d = nc.dram_tensor("xq", [P, 4, KC, 256], bf16, kind="ExternalInput").ap()
    wq_d = nc.dram_tensor("wq", [P, KC, DA], bf16, kind="ExternalInput").ap()
    wk_d = nc.dram_tensor("wk", [P, KC, DA], bf16, kind="ExternalInput").ap()
    wv_d = nc.dram_tensor("wv", [P, 2, KC, 512], bf16, kind="ExternalInput").ap()
    kmb = nc.dram_tensor("kmb", [P, 1536], bf16, kind="ExternalInput").ap()
    qmn = nc.dram_tensor("qmn", [P, 8], f32, kind="ExternalInput").ap()
    # packed per-slot final-chunk masks: even slots are 256 wide, odd 512
    dmask = nc.dram_tensor("dmask", [P, 3072], bf16, kind="ExternalInput").ap()

    outT = nc.dram_tensor("outT", [D, NQ], bf16, kind="ExternalOutput").ap()
    den = nc.dram_tensor("den", [P, 8], f32, kind="ExternalOutput").ap()

    xb_r = xb.rearrange("(g p) d -> p g d", p=P)
    outT_r = outT.rearrange("(do p) q -> p do q", p=P)

    with tile.TileContext(nc) as tc, ExitStack() as ctx:
        const = ctx.enter_context(tc.tile_pool(name="const", bufs=1))
        persist = ctx.enter_context(tc.tile_pool(name="persist", bufs=1))
        stream = ctx.enter_context(tc.tile_pool(name="stream", bufs=2))
        apool = ctx.enter_context(tc.tile_pool(name="apool", bufs=4))
        epool = ctx.enter_context(tc.tile_pool(name="epool", bufs=16))
        psl_pool = ctx.enter_context(tc.tile_pool(name="psl", bufs=2, space="PSUM"))
        psT_pool = ctx.enter_context(tc.tile_pool(name="psT", bufs=2, space="PSUM"))
        psax_pool = ctx.enter_context(tc.tile_pool(name="psax", bufs=2, space="PSUM"))
        pjp_pool = ctx.enter_context(tc.tile_pool(name="pjp", bufs=2, space="PSUM"))

        ident_f32 = const.tile([P, P], f32, name="ident_f32")
        make_identity(nc, ident_f32)
        ident = const.tile([P, P], bf16, name="ident")
        nc.vector.tensor_copy(ident[:], ident_f32[:])

        wq_sb = const.tile([P, KC, DA], bf16, name="wq_sb")
        wk_sb = const.tile([P, KC, DA], bf16, name="wk_sb")
        wv_sb = const.tile([P, 2, KC, 512], bf16, name="wv_sb")
        kmb_sb = const.tile([P, 1536], bf16, name="kmb_sb")
        qmn_sb = const.tile([P, 8], f32, name="qmn_sb")
        dm_sb = const.tile([P, 3072], bf16, name="dm_sb")
        # packed column offsets of slot s's final-chunk mask in dm_sb
        DOFF = [0, 256, 768, 1024, 1536, 1792, 2304, 2560]

        kT_sb = persist.tile([P, S], bf16, name="kT_sb")
        qT_sb = persist.tile([P, NQ], bf16, name="qT_sb")
        xb_sb = persist.tile([P, 16, D], bf16, name="xb_sb")
        den_sb = persist.tile([P, 8], f32, name="den_sb")

        es: dict = {}
        dacs: dict = {s: [] for s in range(8)}

        def qT_quarter(qtr, nsub=1):
            xq = stream.tile([P, KC, 256], bf16, tag="xq", name=f"xq{qtr}")
            kcs = KC // nsub
            for sub in range(nsub):
                if qtr == 0:
                    nc.sync.dma_start(
                        wq_sb[:, sub * kcs:(sub + 1) * kcs, :],
                        wq_d[:, sub * kcs:(sub + 1) * kcs, :])
                nc.sync.dma_start(
                    xq[:, sub * kcs:(sub + 1) * kcs, :],
                    xq_d[:, qtr, sub * kcs:(sub + 1) * kcs, :])
            ps = pjp_pool.tile([P, 512], f32, tag="pjp", name=f"psq{qtr}")
            for kc in range(KC):
                nc.tensor.matmul(
                    ps[:, :256],
                    lhsT=wq_sb[:, kc, :],
                    rhs=xq[:, kc, :],
                    start=(kc == 0), stop=(kc == KC - 1))
            nc.any.tensor_copy(qT_sb[:, qtr * 256:(qtr + 1) * 256], ps[:, :256])

        def kT_chunk(j, nsub=1, wk_nsub=2):
            ct = stream.tile([P, KC, 512], bf16, tag="ct", name=f"ct{j}")
            kcs = KC // nsub
            wkcs = KC // wk_nsub
            for sub in range(nsub):
                if j == 0 and sub % (nsub // wk_nsub) == 0:
                    wsub = sub // (nsub // wk_nsub)
                    nc.sync.dma_start(
                        wk_sb[:, wsub * wkcs:(wsub + 1) * wkcs, :],
                        wk_d[:, wsub * wkcs:(wsub + 1) * wkcs, :])
                nc.sync.dma_start(
                    ct[:, sub * kcs:(sub + 1) * kcs, :],
                    ct_d[:, j, sub * kcs:(sub + 1) * kcs, :])
            ps = pjp_pool.tile([P, 512], f32, tag="pjp", name=f"psk{j}")
            for kc in range(KC):
                nc.tensor.matmul(
                    ps[:],
                    lhsT=wk_sb[:, kc, :],
                    rhs=ct[:, kc, :],
                    start=(kc == 0), stop=(kc == KC - 1))
            nc.any.tensor_copy(kT_sb[:, j * 512:(j + 1) * 512], ps[:])

        def pair_logits(pr):
            for j in range(pr + 1):
                for s in (2 * pr, 2 * pr + 1):
                    last = (j == pr)
                    w = 256 if (last and s % 2 == 0) else 512
                    off = j * 512
                    psl = psl_pool.tile([P, 512], f32, tag="psl",
                                        name=f"psl{s}_{j}")
                    nc.tensor.matmul(
                        psl[:, :w],
                        lhsT=qT_sb[:, s * P:(s + 1) * P],
                        rhs=kT_sb[:, off:off + w],
                        start=True, stop=True)
                    sbl = apool.tile([P, 512], f32, tag="sbl",
                                     name=f"sbl{s}_{j}", bufs=8)
                    msrc = dm_sb[:, DOFF[s]:DOFF[s] + w] if last \
                        else kmb_sb[:, off:off + w]
                    nc.vector.tensor_tensor(
                        out=sbl[:, :w], in0=psl[:, :w], in1=msrc, op=ALU.add)
                    e = apool.tile([P, 512], bf16, tag="e",
                                   name=f"e{s}_{j}", bufs=10)
                    dac = apool.tile([P, 1], f32, tag="dac",
                                     name=f"dac{s}_{j}", bufs=12)
                    nc.scalar.activation(
                        e[:, :w], sbl[:, :w], AF.Exp,
                        bias=qmn_sb[:, s:s + 1], scale=1.0,
                        accum_out=dac[:])
                    es[(s, j)] = e
                    dacs[s].append(dac)

        def pair_transp(pr):
            # two key-blocks share one [P,512] psT bank and one eT tile:
            # halves the psum->sbuf copy count (each copy's ~1us sem
            # turnaround is the transpose phase's limiter, not PE time)
            eTs = []
            s0, s1 = 2 * pr, 2 * pr + 1
            for j in range(pr + 1):
                last = (j == pr)
                for kh in range(2):
                    psT = psT_pool.tile([P, 512], bf16, tag="psT",
                                        name=f"psT{pr}_{j}_{kh}")
                    eT = epool.tile([P, 512], bf16, tag="eT",
                                    name=f"eT{pr}_{j}_{kh}")
                    tails = last and kh == 1
                    for ki in range(2):
                        ks = 2 * kh + ki
                        base = ki * 256
                        if tails:
                            nc.tensor.transpose(
                                psT[:, base:base + P],
                                es[(s1, j)][:, ks * P:(ks + 1) * P],
                                ident[:])
                        else:
                            nc.tensor.transpose(
                                psT[:, base:base + P],
                                es[(s0, j)][:, ks * P:(ks + 1) * P],
                                ident[:])
                            nc.tensor.transpose(
                                psT[:, base + P:base + 2 * P],
                                es[(s1, j)][:, ks * P:(ks + 1) * P],
                                ident[:])
                    if tails:
                        # only cols [0:128] and [256:384] were written
                        nc.any.tensor_copy(eT[:, 0:P], psT[:, 0:P])
                        nc.any.tensor_copy(eT[:, 256:256 + P],
                                           psT[:, 256:256 + P])
                    else:
                        nc.any.tensor_copy(eT[:], psT[:])
                    for ki in range(2):
                        eTs.append((eT, ki * 256, not tails))
            return eTs

        def pair_ax(pr, eTs):
            nkb0 = 4 * pr + 2
            nkb1 = 4 * pr + 4
            zT = apool.tile([P, KC, 256], bf16, tag="zT", name=f"zT{pr}",
                            bufs=4)
            # two dm-chunks share one [128,512] bank as a single
            # accumulation group (start lazily zeroes the whole 2KB
            # region), so one copy drains two dm-chunks: the copy
            # turnaround (~1us with sem latency) stays ahead of the
            # ~1.3us 2-chunk matmul stream on 2 rotating banks
            for dmh in range(KC // 2):
                ps = psax_pool.tile([P, 512], f32, tag="psax",
                                    name=f"psax{pr}_{dmh}")
                for half in range(2):
                    dmc = dmh * 2 + half
                    base = half * 256
                    for kb in range(nkb1):
                        eT, eb, both = eTs[kb]
                        lhsT = xb_sb[:, kb, dmc * P:(dmc + 1) * P]
                        first = (kb == 0 and half == 0)
                        last = (kb == nkb1 - 1 and half == 1)
                        if both:
                            nc.tensor.matmul(
                                ps[:, base:base + P], lhsT=lhsT,
                                rhs=eT[:, eb:eb + P],
                                start=first, stop=False)
                            nc.tensor.matmul(
                                ps[:, base + P:base + 2 * P], lhsT=lhsT,
                                rhs=eT[:, eb + P:eb + 2 * P],
                                start=False, stop=False)
                        else:
                            nc.tensor.matmul(
                                ps[:, base + P:base + 2 * P], lhsT=lhsT,
                                rhs=eT[:, eb:eb + P],
                                start=False, stop=last)
                nc.any.tensor_copy(zT[:, dmh * 2:dmh * 2 + 2, :], ps[:])
            return zT

        def pair_proj(pr, zT):
            osb = apool.tile([P, KC, 256], bf16, tag="osb", name=f"osb{pr}",
                             bufs=2)
            # outputs ride the ACT HWDGE queue: the SP input FIFO never
            # stalls behind an output whose source copy isn't done yet.
            # The last pair flushes per dout so the final DMA after the
            # last matmul is only 64KB.
            ndma = 8 if pr == 3 else 1
            dos = KC // ndma
            for do in range(KC):
                ps = pjp_pool.tile([P, 512], f32, tag="pjp",
                                   name=f"psp{pr}_{do}")
                for dmc in range(KC):
                    nc.tensor.matmul(
                        ps[:, :256],
                        lhsT=wv_sb[:, do // 4, dmc,
                                   (do % 4) * P:(do % 4 + 1) * P],
                        rhs=zT[:, dmc, :],
                        start=(dmc == 0), stop=(dmc == KC - 1))
                nc.any.tensor_copy(osb[:, do, :], ps[:, :256])
                if do % dos == dos - 1:
                    g0 = do - dos + 1
                    # the very last piece rides the (idle) SP queue: its
                    # DGE delay is 650ns vs the ACT queue's 784ns
                    eng = nc.sync if (pr == 3 and do == KC - 1) else nc.scalar
                    eng.dma_start(
                        outT_r[:, g0:do + 1, pr * 256:(pr + 1) * 256],
                        osb[:, g0:do + 1, :])

        def pair_den(pr):
            for s in (2 * pr, 2 * pr + 1):
                dl = dacs[s]
                dst = den_sb[:, s:s + 1]
                if len(dl) == 1:
                    nc.vector.tensor_copy(dst, dl[0][:])
                else:
                    nc.vector.tensor_tensor(out=dst, in0=dl[0][:],
                                            in1=dl[1][:], op=ALU.add)
                    for d in dl[2:]:
                        nc.vector.tensor_tensor(out=dst, in0=dst, in1=d[:],
                                                op=ALU.add)

        # ---- schedule ----
        # Software-pipelined PE stream; SP FIFO carries inputs in exactly
        # the PE consumption order:
        #   kT0 qT0 log0 qT1 T0 | kT1 log1 AX0 qT2 T1 | kT2 log2 AX1 qT3
        #   T2 | kT3 log3 AX2 T3 AX3 | proj0..3
        # AX(pr-1) + qT(pr+1) fill the exp(pr) DVE+ACT latency before
        # T(pr); all Wv projections run dependency-free at the tail.
        def dm_piece(pr):
            nc.sync.dma_start(dm_sb[:, 768 * pr:768 * (pr + 1)],
                              dmask[:, 768 * pr:768 * (pr + 1)])

        zTs = {}
        eT_all = {}

        kT_chunk(0, nsub=2)
        qT_quarter(0, nsub=2)
        nc.sync.dma_start(qmn_sb[:], qmn)
        dm_piece(0)
        pair_logits(0)
        qT_quarter(1)
        eT_all[0] = pair_transp(0)
        pair_den(0)

        kT_chunk(1)
        nc.sync.dma_start(xb_sb[:, 0:4, :], xb_r[:, 0:4, :])
        nc.sync.dma_start(kmb_sb[:], kmb)
        dm_piece(1)
        pair_logits(1)
        zTs[0] = pair_ax(0, eT_all[0])
        eT_all[1] = pair_transp(1)
        qT_quarter(2)
        pair_den(1)

        kT_chunk(2)
        nc.sync.dma_start(xb_sb[:, 4:8, :], xb_r[:, 4:8, :])
        dm_piece(2)
        pair_logits(2)
        zTs[1] = pair_ax(1, eT_all[1])
        eT_all[2] = pair_transp(2)
        qT_quarter(3)
        pair_den(2)

        kT_chunk(3)
        nc.sync.dma_start(xb_sb[:, 8:12, :], xb_r[:, 8:12, :])
        dm_piece(3)
        nc.sync.dma_start(xb_sb[:, 12:16, :], xb_r[:, 12:16, :])
        # wv is only consumed by the tail projections; keeping it out of
        # the front window lets xb/ct/xq land sooner
        nc.sync.dma_start(wv_sb[:, 0], wv_d[:, 0])
        nc.sync.dma_start(wv_sb[:, 1], wv_d[:, 1])
        pair_logits(3)
        zTs[2] = pair_ax(2, eT_all[2])
        eT_all[3] = pair_transp(3)
        pair_den(3)
        # den is final here; flush it during the tail work instead of
        # after the last output piece
        nc.scalar.dma_start(den[:], den_sb[:])
        zTs[3] = pair_ax(3, eT_all[3])
        for pr in range(4):
            pair_proj(pr, zTs[pr])

    nc.compile()
    return nc


def _get_nc():
    with _BUILD_LOCK:
        if "nc" not in _CACHE:
            _CACHE["nc"] = _build()
        return _CACHE["nc"]


def make_in_maps(x, cross, Wq, Wk, Wv, mask):
    bf = ml_dtypes.bfloat16
    x = np.asarray(x, dtype=np.float32)
    cross = np.asarray(cross, dtype=np.float32)
    scale = 1.0 / math.sqrt(DA)
    # weight layouts pre-rearranged for partition-contiguous DMA rows
    # wq/wk: [dm, da] -> [p, kc, da]  (dm = kc*128 + p)
    wq_h = np.ascontiguousarray(
        (np.asarray(Wq, np.float32) * scale).T.reshape(KC, P, DA)
        .transpose(1, 0, 2)).astype(bf)
    wk_h = np.ascontiguousarray(
        np.asarray(Wk, np.float32).T.reshape(KC, P, DA)
        .transpose(1, 0, 2)).astype(bf)
    # wv: [dm, dout] -> [p, half, kc, 512]
    wv_h = np.ascontiguousarray(
        np.asarray(Wv, np.float32).T.reshape(KC, P, 2, 512)
        .transpose(1, 2, 0, 3)).astype(bf)
    mf = np.asarray(mask).astype(np.float32)  # [B, S]

    karange = np.arange(S)
    in_maps = []
    rows_per_core = []
    for core in range(NCORES):
        b, p = divmod(core, 2)
        blocks = STRIPS[p]
        rows = np.concatenate([np.arange(g * P, (g + 1) * P) for g in blocks])
        rows_per_core.append((b, rows))
        mb = mf[b]
        kneg = (-BIG * (1.0 - mb)).astype(np.float32)  # [S]
        kmb_h = np.ascontiguousarray(
            np.broadcast_to(kneg[:1536], (P, 1536))).astype(bf)
        mq = mb[rows]  # [1024]
        qmn_h = np.ascontiguousarray(
            (-BIG * (1.0 - mq)).reshape(8, P).T)  # [128, 8]
        # packed per-slot final-chunk masks (even slots 256 wide, odd 512)
        doff = [0, 256, 768, 1024, 1536, 1792, 2304, 2560]
        dm_h = np.zeros((P, 3072), np.float32)
        for s, g in enumerate(blocks):
            pr = s // 2
            k0 = 512 * pr
            w = 256 if s % 2 == 0 else 512
            kk = karange[k0:k0 + w]
            qq = g * P + np.arange(P)
            mqs = mq[s * P:(s + 1) * P]
            t = np.broadcast_to(kneg[k0:k0 + w], (P, w)).copy()
            t += -BIG * (kk[None, :] > qq[:, None])
            # diagonal is ALWAYS kept (reference adds eye before the >0
            # test): cancel qmn's -BIG for masked q rows, zero otherwise
            dd = kk[None, :] == qq[:, None]
            t = np.where(dd, (BIG * (1.0 - mqs))[:, None], t)
            dm_h[:, doff[s]:doff[s] + w] = t
        # cross^T: [dm, key] -> [p, chunk_j, kc, 512]
        ct_h = np.ascontiguousarray(
            cross[b].T.reshape(KC, P, 4, 512).transpose(1, 2, 0, 3)).astype(bf)
        # x[rows]^T: [dm, q] -> [p, qtr, kc, 256]
        xq_h = np.ascontiguousarray(
            x[b][rows].T.reshape(KC, P, 4, 256).transpose(1, 2, 0, 3)).astype(bf)
        in_maps.append({
            "xb": np.ascontiguousarray(x[b]).astype(bf),
            "ct": ct_h,
            "xq": xq_h,
            "wq": wq_h,
            "wk": wk_h,
            "wv": wv_h,
            "kmb": kmb_h,
            "qmn": qmn_h,
            "dmask": dm_h.astype(bf),
        })
    return in_maps, rows_per_core


def kernel(x, cross, Wq, Wk, Wv, mask):
    from concourse import bass_utils

    nc = _get_nc()
    in_maps, rows_per_core = make_in_maps(x, cross, Wq, Wk, Wv, mask)
    res = bass_utils.run_bass_kernel_spmd(
        nc, in_maps, core_ids=list(range(NCORES)))

    out = np.empty((B, S, D), np.float32)
    for core in range(NCORES):
        b, rows = rows_per_core[core]
        r = res.results[core]
        o = r["outT"].astype(np.float32).T  # [1024 q, 1024 dm]
        denf = r["den"].T.reshape(-1).astype(np.float32)  # [1024] slot-major
        out[b, rows] = o / denf[:, None]
    return out


# revision 5
# speedup vs baseline: 1.0235x; 1.0118x over previous
"""Trainium2 Bass kernel v2 for nn_Attention_42288247996512.

reference:
  q = x @ Wq.T; k = cross @ Wk.T; v = x @ Wv.T
  logits = q @ k.T  (causal; padding m_q*m_k, diag always kept)
  out = softmax(logits / sqrt(128)) @ v

v2 refactor vs baseline:
  * out^T = Wv @ (x^T @ attn^T): the value projection moves to the query
    side (65536 PE rows/core) instead of projecting v for all 2048 keys
    (131072 rows). The AX stage (x^T @ attn^T) consumes raw x in natural
    layout as lhsT -- no projection needed before it.
  * all matmuls in bf16 (1.0 PE cycles/row at any ap size, transposes
    1.0 vs f32r's 1.5), all DMA payloads bf16 (halves HBM traffic).
  * strips interleave even/odd key blocks so both strips share exact
    per-slot causal widths W[s] = 256*(s+1): logits/exp/transpose/AX run
    only over live key blocks (73728 AX rows vs 81920 for 512-chunking).
  * single SP HWDGE FIFO carries inputs in consumption order; outputs
    ride the ACT HWDGE queue so they never block input delivery.

Per-core PE rows: qT 8192 + kT 16384 + logits 9216 + transp 9216 +
AX 73728 + proj 65536 = 182272 (~76us at peak rate) vs ~263k in v1.
"""
import math
import threading

import ml_dtypes
import numpy as np

B, S, D, DA = 4, 2048, 1024, 128
P = 128
KC = D // P  # 8 contraction chunks
NCORES = 8
NQ = 1024
BIG = 32768.0  # power of two: exactly representable in bf16

# Two per-parity programs (one compiled kernel per strip) let each strip
# pay only its own causal widths: both splits below sum to 68 key-block
# units vs 72 for the best single-SPMD assignment (even/odd interleave).
# Chunk counts, kT/xb/qT DMA alignment and the schedule skeleton are
# identical across both programs; only nkb and last-chunk widths differ.
STRIPS = [[0, 3, 4, 7, 8, 11, 12, 15], [1, 2, 5, 6, 9, 10, 13, 14]]

_BUILD_LOCK = threading.Lock()
_CACHE: dict = {}


def _build(blocks):
    from contextlib import ExitStack

    import concourse.mybir as mybir
    import concourse.tile as tile
    from concourse import bacc
    from concourse.masks import make_identity

    dt = mybir.dt
    f32 = dt.float32
    bf16 = dt.bfloat16
    AF = mybir.ActivationFunctionType
    ALU = mybir.AluOpType

    # per-pair key-block counts and last-chunk widths from this strip's
    # block assignment (slots sorted ascending; pairs are adjacent slots)
    NKB = [(blocks[2 * i] + 1, blocks[2 * i + 1] + 1) for i in range(4)]
    WL = [(NKB[i][0] * 128 - 512 * i, NKB[i][1] * 128 - 512 * i)
          for i in range(4)]

    nc = bacc.Bacc("TRN2", target_bir_lowering=False, debug=False)

    # all inputs pre-rearranged on host so every DMA source row is
    # partition-contiguous (2KB+ descriptors run at ~360B/ns; the naive
    # (kc p) m -> p kc m rearranges produce 256B descriptors at ~172B/ns)
    xb = nc.dram_tensor("xb", [S, D], bf16, kind="ExternalInput").ap()

    ct_d = nc.dram_tensor("ct", [P, 4, KC, 512], bf16, kind="ExternalInput").ap()
    xq_d = nc.dram_tensor("xq", [P, 4, KC, 256], bf16, kind="ExternalInput").ap()
    wq_d = nc.dram_tensor("wq", [P, KC, DA], bf16, kind="ExternalInput").ap()
    wk_d = nc.dram_tensor("wk", [P, KC, DA], bf16, kind="ExternalInput").ap()
    wv_d = nc.dram_tensor("wv", [P, 2, KC, 512], bf16, kind="ExternalInput").ap()
    kmb = nc.dram_tensor("kmb", [P, 1536], bf16, kind="ExternalInput").ap()
    qmn = nc.dram_tensor("qmn", [P, 8], f32, kind="ExternalInput").ap()
    # packed per-slot final-chunk masks: even slots are 256 wide, odd 512
    dmask = nc.dram_tensor("dmask", [P, 2560], bf16, kind="ExternalInput").ap()

    outT = nc.dram_tensor("outT", [D, NQ], bf16, kind="ExternalOutput").ap()
    den = nc.dram_tensor("den", [P, 8], f32, kind="ExternalOutput").ap()

    xb_r = xb.rearrange("(g p) d -> p g d", p=P)
    outT_r = outT.rearrange("(do p) q -> p do q", p=P)

    with tile.TileContext(nc) as tc, ExitStack() as ctx:
        const = ctx.enter_context(tc.tile_pool(name="const", bufs=1))
        persist = ctx.enter_context(tc.tile_pool(name="persist", bufs=1))
        stream = ctx.enter_context(tc.tile_pool(name="stream", bufs=2))
        apool = ctx.enter_context(tc.tile_pool(name="apool", bufs=4))
        epool = ctx.enter_context(tc.tile_pool(name="epool", bufs=16))
        psl_pool = ctx.enter_context(tc.tile_pool(name="psl", bufs=2, space="PSUM"))
        psT_pool = ctx.enter_context(tc.tile_pool(name="psT", bufs=2, space="PSUM"))
        psax_pool = ctx.enter_context(tc.tile_pool(name="psax", bufs=2, space="PSUM"))
        pjp_pool = ctx.enter_context(tc.tile_pool(name="pjp", bufs=2, space="PSUM"))

        ident_f32 = const.tile([P, P], f32, name="ident_f32")
        make_identity(nc, ident_f32)
        ident = const.tile([P, P], bf16, name="ident")
        nc.vector.tensor_copy(ident[:], ident_f32[:])

        wq_sb = const.tile([P, KC, DA], bf16, name="wq_sb")
        wk_sb = const.tile([P, KC, DA], bf16, name="wk_sb")
        wv_sb = const.tile([P, 2, KC, 512], bf16, name="wv_sb")
        kmb_sb = const.tile([P, 1536], bf16, name="kmb_sb")
        qmn_sb = const.tile([P, 8], f32, name="qmn_sb")
        dm_sb = const.tile([P, 2560], bf16, name="dm_sb")
        # packed column offsets of slot s's final-chunk mask in dm_sb
        DOFF = [640 * (s // 2) + (0 if s % 2 == 0 else WL[s // 2][0])
                for s in range(8)]

        kT_sb = persist.tile([P, S], bf16, name="kT_sb")
        qT_sb = persist.tile([P, NQ], bf16, name="qT_sb")
        xb_sb = persist.tile([P, 16, D], bf16, name="xb_sb")
        den_sb = persist.tile([P, 8], f32, name="den_sb")

        es: dict = {}
        dacs: dict = {s: [] for s in range(8)}

        def qT_quarter(qtr, nsub=1):
            xq = stream.tile([P, KC, 256], bf16, tag="xq", name=f"xq{qtr}")
            kcs = KC // nsub
            for sub in range(nsub):
                if qtr == 0:
                    nc.sync.dma_start(
                        wq_sb[:, sub * kcs:(sub + 1) * kcs, :],
                        wq_d[:, sub * kcs:(sub + 1) * kcs, :])
                nc.sync.dma_start(
                    xq[:, sub * kcs:(sub + 1) * kcs, :],
                    xq_d[:, qtr, sub * kcs:(sub + 1) * kcs, :])
            ps = pjp_pool.tile([P, 512], f32, tag="pjp", name=f"psq{qtr}")
            for kc in range(KC):
                nc.tensor.matmul(
                    ps[:, :256],
                    lhsT=wq_sb[:, kc, :],
                    rhs=xq[:, kc, :],
                    start=(kc == 0), stop=(kc == KC - 1))
            nc.any.tensor_copy(qT_sb[:, qtr * 256:(qtr + 1) * 256], ps[:, :256])

        def kT_chunk(j, nsub=1, wk_nsub=2):
            ct = stream.tile([P, KC, 512], bf16, tag="ct", name=f"ct{j}")
            kcs = KC // nsub
            wkcs = KC // wk_nsub
            for sub in range(nsub):
                if j == 0 and sub % (nsub // wk_nsub) == 0:
                    wsub = sub // (nsub // wk_nsub)
                    nc.sync.dma_start(
                        wk_sb[:, wsub * wkcs:(wsub + 1) * wkcs, :],
                        wk_d[:, wsub * wkcs:(wsub + 1) * wkcs, :])
                nc.sync.dma_start(
                    ct[:, sub * kcs:(sub + 1) * kcs, :],
                    ct_d[:, j, sub * kcs:(sub + 1) * kcs, :])
            ps = pjp_pool.tile([P, 512], f32, tag="pjp", name=f"psk{j}")
            for kc in range(KC):
                nc.tensor.matmul(
                    ps[:],
                    lhsT=wk_sb[:, kc, :],
                    rhs=ct[:, kc, :],
                    start=(kc == 0), stop=(kc == KC - 1))
            nc.any.tensor_copy(kT_sb[:, j * 512:(j + 1) * 512], ps[:])

        def pair_logits(pr):
            for j in range(pr + 1):
                for s in (2 * pr, 2 * pr + 1):
                    last = (j == pr)
                    w = WL[pr][s % 2] if last else 512
                    off = j * 512
                    psl = psl_pool.tile([P, 512], f32, tag="psl",
                                        name=f"psl{s}_{j}")
                    nc.tensor.matmul(
                        psl[:, :w],
                        lhsT=qT_sb[:, s * P:(s + 1) * P],
                        rhs=kT_sb[:, off:off + w],
                        start=True, stop=True)
                    sbl = apool.tile([P, 512], f32, tag="sbl",
                                     name=f"sbl{s}_{j}", bufs=8)
                    msrc = dm_sb[:, DOFF[s]:DOFF[s] + w] if last \
                        else kmb_sb[:, off:off + w]
                    nc.vector.tensor_tensor(
                        out=sbl[:, :w], in0=psl[:, :w], in1=msrc, op=ALU.add)
                    e = apool.tile([P, 512], bf16, tag="e",
                                   name=f"e{s}_{j}", bufs=10)
                    dac = apool.tile([P, 1], f32, tag="dac",
                                     name=f"dac{s}_{j}", bufs=12)
                    nc.scalar.activation(
                        e[:, :w], sbl[:, :w], AF.Exp,
                        bias=qmn_sb[:, s:s + 1], scale=1.0,
                        accum_out=dac[:])
                    es[(s, j)] = e
                    dacs[s].append(dac)

        def pair_transp(pr):
            # two key-blocks share one [P,512] psT bank and one eT tile:
            # halves the psum->sbuf copy count (each copy's ~1us sem
            # turnaround is the transpose phase's limiter, not PE time).
            # kb < nkb0: both slots live ([P,256] per kb); else only the
            # odd slot ([P,128]). Copies cover exactly the written runs.
            eTs = []
            s0, s1 = 2 * pr, 2 * pr + 1
            nkb0, nkb1 = NKB[pr]
            for kh in range((nkb1 + 1) // 2):
                psT = psT_pool.tile([P, 512], bf16, tag="psT",
                                    name=f"psT{pr}_{kh}")
                eT = epool.tile([P, 512], bf16, tag="eT",
                                name=f"eT{pr}_{kh}")
                runs = []
                for ki in range(2):
                    kb = 2 * kh + ki
                    if kb >= nkb1:
                        break
                    base = ki * 256
                    j, ks = kb // 4, kb % 4
                    if kb < nkb0:
                        nc.tensor.transpose(
                            psT[:, base:base + P],
                            es[(s0, j)][:, ks * P:(ks + 1) * P],
                            ident[:])
                        nc.tensor.transpose(
                            psT[:, base + P:base + 2 * P],
                            es[(s1, j)][:, ks * P:(ks + 1) * P],
                            ident[:])
                        w = 256
                    else:
                        nc.tensor.transpose(
                            psT[:, base:base + P],
                            es[(s1, j)][:, ks * P:(ks + 1) * P],
                            ident[:])
                        w = 128
                    if runs and runs[-1][0] + runs[-1][1] == base:
                        runs[-1] = (runs[-1][0], runs[-1][1] + w)
                    else:
                        runs.append((base, w))
                    eTs.append((eT, base, kb < nkb0))
                for b, w in runs:
                    nc.any.tensor_copy(eT[:, b:b + w], psT[:, b:b + w])
            return eTs

        def pair_ax(pr, eTs):
            nkb0, nkb1 = NKB[pr]
            zT = apool.tile([P, KC, 256], bf16, tag="zT", name=f"zT{pr}",
                            bufs=4)
            # two dm-chunks share one [128,512] bank as a single
            # accumulation group (start lazily zeroes the whole 2KB
            # region), so one copy drains two dm-chunks: the copy
            # turnaround (~1us with sem latency) stays ahead of the
            # ~1.3us 2-chunk matmul stream on 2 rotating banks
            for dmh in range(KC // 2):
                ps = psax_pool.tile([P, 512], f32, tag="psax",
                                    name=f"psax{pr}_{dmh}")
                for half in range(2):
                    dmc = dmh * 2 + half
                    base = half * 256
                    for kb in range(nkb1):
                        eT, eb, both = eTs[kb]
                        lhsT = xb_sb[:, kb, dmc * P:(dmc + 1) * P]
                        first = (kb == 0 and half == 0)
                        last = (kb == nkb1 - 1 and half == 1)
                        if both:
                            nc.tensor.matmul(
                                ps[:, base:base + P], lhsT=lhsT,
                                rhs=eT[:, eb:eb + P],
                                start=first, stop=False)
                            nc.tensor.matmul(
                                ps[:, base + P:base + 2 * P], lhsT=lhsT,
                                rhs=eT[:, eb + P:eb + 2 * P],
                                start=False, stop=False)
                        else:
                            nc.tensor.matmul(
                                ps[:, base + P:base + 2 * P], lhsT=lhsT,
                                rhs=eT[:, eb:eb + P],
                                start=False, stop=last)
                nc.any.tensor_copy(zT[:, dmh * 2:dmh * 2 + 2, :], ps[:])
            return zT

        def pair_proj(pr, zT):
            osb = apool.tile([P, KC, 256], bf16, tag="osb", name=f"osb{pr}",
                             bufs=2)
            # outputs ride the ACT HWDGE queue: the SP input FIFO never
            # stalls behind an output whose source copy isn't done yet.
            # The last pair flushes per dout so the final DMA after the
            # last matmul is only 64KB.
            ndma = 8 if pr == 3 else 1
            dos = KC // ndma
            for do in range(KC):
                ps = pjp_pool.tile([P, 512], f32, tag="pjp",
                                   name=f"psp{pr}_{do}")
                for dmc in range(KC):
                    nc.tensor.matmul(
                        ps[:, :256],
                        lhsT=wv_sb[:, do // 4, dmc,
                                   (do % 4) * P:(do % 4 + 1) * P],
                        rhs=zT[:, dmc, :],
                        start=(dmc == 0), stop=(dmc == KC - 1))
                nc.any.tensor_copy(osb[:, do, :], ps[:, :256])
                if do % dos == dos - 1:
                    g0 = do - dos + 1
                    # the very last piece rides the (idle) SP queue: its
                    # DGE delay is 650ns vs the ACT queue's 784ns
                    eng = nc.sync if (pr == 3 and do == KC - 1) else nc.scalar
                    eng.dma_start(
                        outT_r[:, g0:do + 1, pr * 256:(pr + 1) * 256],
                        osb[:, g0:do + 1, :])

        def pair_den(pr):
            for s in (2 * pr, 2 * pr + 1):
                dl = dacs[s]
                dst = den_sb[:, s:s + 1]
                if len(dl) == 1:
                    nc.vector.tensor_copy(dst, dl[0][:])
                else:
                    nc.vector.tensor_tensor(out=dst, in0=dl[0][:],
                                            in1=dl[1][:], op=ALU.add)
                    for d in dl[2:]:
                        nc.vector.tensor_tensor(out=dst, in0=dst, in1=d[:],
                                                op=ALU.add)

        # ---- schedule ----
        # Software-pipelined PE stream; SP FIFO carries inputs in exactly
        # the PE consumption order:
        #   kT0 qT0 log0 qT1 T0 | kT1 log1 AX0 qT2 T1 | kT2 log2 AX1 qT3
        #   T2 | kT3 log3 AX2 T3 AX3 | proj0..3
        # AX(pr-1) + qT(pr+1) fill the exp(pr) DVE+ACT latency before
        # T(pr); all Wv projections run dependency-free at the tail.
        def dm_piece(pr):
            nc.sync.dma_start(dm_sb[:, 640 * pr:640 * (pr + 1)],
                              dmask[:, 640 * pr:640 * (pr + 1)])

        zTs = {}
        eT_all = {}

        kT_chunk(0, nsub=2)
        qT_quarter(0, nsub=2)
        nc.sync.dma_start(qmn_sb[:], qmn)
        dm_piece(0)
        pair_logits(0)
        qT_quarter(1)
        eT_all[0] = pair_transp(0)
        pair_den(0)

        kT_chunk(1)
        nc.sync.dma_start(xb_sb[:, 0:4, :], xb_r[:, 0:4, :])
        nc.sync.dma_start(kmb_sb[:], kmb)
        dm_piece(1)
        pair_logits(1)
        zTs[0] = pair_ax(0, eT_all[0])
        eT_all[1] = pair_transp(1)
        qT_quarter(2)
        pair_den(1)

        kT_chunk(2)
        nc.sync.dma_start(xb_sb[:, 4:8, :], xb_r[:, 4:8, :])
        dm_piece(2)
        pair_logits(2)
        zTs[1] = pair_ax(1, eT_all[1])
        eT_all[2] = pair_transp(2)
        qT_quarter(3)
        pair_den(2)

        kT_chunk(3)
        nc.sync.dma_start(xb_sb[:, 8:12, :], xb_r[:, 8:12, :])
        dm_piece(3)
        nc.sync.dma_start(xb_sb[:, 12:16, :], xb_r[:, 12:16, :])
        # wv is only consumed by the tail projections; keeping it out of
        # the front window lets xb/ct/xq land sooner
        nc.sync.dma_start(wv_sb[:, 0], wv_d[:, 0])
        nc.sync.dma_start(wv_sb[:, 1], wv_d[:, 1])
        pair_logits(3)
        zTs[2] = pair_ax(2, eT_all[2])
        eT_all[3] = pair_transp(3)
        pair_den(3)
        # den is final here; flush it during the tail work instead of
        # after the last output piece
        nc.scalar.dma_start(den[:], den_sb[:])
        zTs[3] = pair_ax(3, eT_all[3])
        for pr in range(4):
            pair_proj(pr, zTs[pr])

    nc.compile()
    return nc


def _get_ncs():
    with _BUILD_LOCK:
        if "ncs" not in _CACHE:
            _CACHE["ncs"] = tuple(_build(bl) for bl in STRIPS)
        return _CACHE["ncs"]


def _get_nc():
    # both per-parity programs are the same size (68 key-block units);
    # program 0 is the representative for timing/introspection
    return _get_ncs()[0]


def make_in_maps(x, cross, Wq, Wk, Wv, mask):
    bf = ml_dtypes.bfloat16
    x = np.asarray(x, dtype=np.float32)
    cross = np.asarray(cross, dtype=np.float32)
    scale = 1.0 / math.sqrt(DA)
    # weight layouts pre-rearranged for partition-contiguous DMA rows
    # wq/wk: [dm, da] -> [p, kc, da]  (dm = kc*128 + p)
    wq_h = np.ascontiguousarray(
        (np.asarray(Wq, np.float32) * scale).T.reshape(KC, P, DA)
        .transpose(1, 0, 2)).astype(bf)
    wk_h = np.ascontiguousarray(
        np.asarray(Wk, np.float32).T.reshape(KC, P, DA)
        .transpose(1, 0, 2)).astype(bf)
    # wv: [dm, dout] -> [p, half, kc, 512]
    wv_h = np.ascontiguousarray(
        np.asarray(Wv, np.float32).T.reshape(KC, P, 2, 512)
        .transpose(1, 2, 0, 3)).astype(bf)
    mf = np.asarray(mask).astype(np.float32)  # [B, S]

    karange = np.arange(S)
    in_maps = []
    rows_per_core = []
    for core in range(NCORES):
        b, p = divmod(core, 2)
        blocks = STRIPS[p]
        rows = np.concatenate([np.arange(g * P, (g + 1) * P) for g in blocks])
        rows_per_core.append((b, rows))
        mb = mf[b]
        kneg = (-BIG * (1.0 - mb)).astype(np.float32)  # [S]
        kmb_h = np.ascontiguousarray(
            np.broadcast_to(kneg[:1536], (P, 1536))).astype(bf)
        mq = mb[rows]  # [1024]
        qmn_h = np.ascontiguousarray(
            (-BIG * (1.0 - mq)).reshape(8, P).T)  # [128, 8]
        # packed per-slot final-chunk masks, widths from this strip's
        # block assignment: w(s) = (g+1)*128 - 512*pr
        wl = [(g + 1) * 128 - 512 * (s // 2) for s, g in enumerate(blocks)]
        doff = [640 * (s // 2) + (0 if s % 2 == 0 else wl[s - 1])
                for s in range(8)]
        dm_h = np.zeros((P, 2560), np.float32)
        for s, g in enumerate(blocks):
            pr = s // 2
            k0 = 512 * pr
            w = wl[s]
            kk = karange[k0:k0 + w]
            qq = g * P + np.arange(P)
            mqs = mq[s * P:(s + 1) * P]
            t = np.broadcast_to(kneg[k0:k0 + w], (P, w)).copy()
            t += -BIG * (kk[None, :] > qq[:, None])
            # diagonal is ALWAYS kept (reference adds eye before the >0
            # test): cancel qmn's -BIG for masked q rows, zero otherwise
            dd = kk[None, :] == qq[:, None]
            t = np.where(dd, (BIG * (1.0 - mqs))[:, None], t)
            dm_h[:, doff[s]:doff[s] + w] = t
        # cross^T: [dm, key] -> [p, chunk_j, kc, 512]
        ct_h = np.ascontiguousarray(
            cross[b].T.reshape(KC, P, 4, 512).transpose(1, 2, 0, 3)).astype(bf)
        # x[rows]^T: [dm, q] -> [p, qtr, kc, 256]
        xq_h = np.ascontiguousarray(
            x[b][rows].T.reshape(KC, P, 4, 256).transpose(1, 2, 0, 3)).astype(bf)
        in_maps.append({
            "xb": np.ascontiguousarray(x[b]).astype(bf),
            "ct": ct_h,
            "xq": xq_h,
            "wq": wq_h,
            "wk": wk_h,
            "wv": wv_h,
            "kmb": kmb_h,
            "qmn": qmn_h,
            "dmask": dm_h.astype(bf),
        })
    return in_maps, rows_per_core


def kernel(x, cross, Wq, Wk, Wv, mask):
    from concourse import bass_utils

    ncA, ncB = _get_ncs()
    in_maps, rows_per_core = make_in_maps(x, cross, Wq, Wk, Wv, mask)
    groups = ([0, 2, 4, 6], [1, 3, 5, 7])
    results = {}
    for nc_, cores in zip((ncA, ncB), groups):
        res = bass_utils.run_bass_kernel_spmd(
            nc_, [in_maps[c] for c in cores], core_ids=cores)
        for i, c in enumerate(cores):
            results[c] = res.results[i]

    out = np.empty((B, S, D), np.float32)
    for core in range(NCORES):
        b, rows = rows_per_core[core]
        r = results[core]
        o = r["outT"].astype(np.float32).T  # [1024 q, 1024 dm]
        denf = r["den"].T.reshape(-1).astype(np.float32)  # [1024] slot-major
        out[b, rows] = o / denf[:, None]
    return out
